# revision 1
# baseline (speedup 1.0000x reference)
"""Grouped GEMM (MoE expert-parallel) Trainium2 kernel.

8 independent problems: (M_i, 2048) @ (2048, 8192), M_i in MS below.
Sharding: tensor-parallel along N across the 8 cores — every core computes
ALL groups restricted to its own 1024-column slice of N. This is perfectly
load-balanced (each core does sum(M_i) x 2048 x 1024 MACs) and lets a single
SPMD NEFF run on all cores with only the b-slice data differing per core.

Matmuls run in float32r (the tensor engine's fast fp32 mode): measured
~2.4e-4 max rel err vs fp32 on K=2048 dot products, at bf16-rate throughput
(~255 ns per 128x128x512 matmul vs 887 ns for plain fp32).

Host-side pre/post-processing (free — not HW time):
  - A matrices are concatenated, transposed and tiled to [ki, mt, ko, mi]
    so each a-tile DMA is one contiguous 8KB run per partition.
  - B slices are tiled to [g, ki, ko, n] (64KB contiguous per partition).
  - Outputs come back as (mt, mi, n) per core and are re-assembled on host.
"""
import numpy as np

import concourse.bacc as bacc
import concourse.mybir as mybir
import concourse.tile as tile
from concourse.bass_utils import run_bass_kernel_spmd

MS = [4096, 1024, 2048, 3072, 512, 1536, 2560, 768]
K = 2048
N = 8192
NCORES = 8
NSH = N // NCORES  # 1024 columns per core
P = 128
KO = K // P  # 16
MTOT = sum(MS)  # 15616
MT = MTOT // P  # 122 m-tiles
NH = NSH // 512  # 2 psum-tile halves per m-tile

_NC_CACHE = {}


def _build(mode="f32r"):
    if mode in _NC_CACHE:
        return _NC_CACHE[mode]
    in_dt = {"f32r": mybir.dt.float32r, "f32": mybir.dt.float32}[mode]

    nc = bacc.Bacc("TRN2", target_bir_lowering=False)
    aT = nc.dram_tensor("aT", [P, MT, KO, P], in_dt, kind="ExternalInput")
    b = nc.dram_tensor("b", [len(MS), P, KO, NSH], in_dt, kind="ExternalInput")
    o = nc.dram_tensor("o", [MT, P, NSH], mybir.dt.float32, kind="ExternalOutput")

    # m-tile index ranges per group
    g_tiles = []
    t0 = 0
    for m in MS:
        nt = m // P
        g_tiles.append(range(t0, t0 + nt))
        t0 += nt
    assert t0 == MT

    with tile.TileContext(nc) as tc:
        with (
            tc.tile_pool(name="bp", bufs=2) as bp,
            tc.tile_pool(name="ap", bufs=4) as ap,
            tc.tile_pool(name="op", bufs=4) as op,
            tc.tile_pool(name="pp", bufs=8, space="PSUM") as pp,
        ):
            for g in range(len(MS)):
                b_sb = bp.tile([P, KO, NSH], in_dt, tag="b")
                nc.sync.dma_start(b_sb, b[g])
                for mt in g_tiles[g]:
                    a_sb = ap.tile([P, KO, P], in_dt, tag="a")
                    nc.sync.dma_start(a_sb, aT[:, mt])
                    o_sb = op.tile([P, NSH], mybir.dt.float32, tag="o")
                    for nh in range(NH):
                        ps = pp.tile([P, 512], mybir.dt.float32, tag="ps")
                        for ko in range(KO):
                            nc.tensor.matmul(
                                ps,
                                a_sb[:, ko, :],
                                b_sb[:, ko, nh * 512 : (nh + 1) * 512],
                                start=(ko == 0),
                                stop=(ko == KO - 1),
                            )
                        nc.vector.tensor_copy(o_sb[:, nh * 512 : (nh + 1) * 512], ps)
                    nc.sync.dma_start(o[mt], o_sb)

    nc.compile()
    _NC_CACHE[mode] = nc
    return nc


def _prep_inputs(a_list, b_list):
    # A: concat groups -> (MTOT, K); tile to [ki, mt, ko, mi]
    a_cat = np.concatenate(a_list, axis=0)
    aT = np.ascontiguousarray(
        a_cat.reshape(MT, P, KO, P).transpose(3, 0, 2, 1)
    )  # [ki, mt, ko, mi]
    in_maps = []
    for c in range(NCORES):
        bc = np.stack(
            [
                np.ascontiguousarray(
                    bg[:, c * NSH : (c + 1) * NSH]
                    .reshape(KO, P, NSH)
                    .transpose(1, 0, 2)
                )
                for bg in b_list
            ],
            axis=0,
        )  # (G, ki, ko, n)
        in_maps.append({"aT": aT, "b": bc})
    return in_maps


def _gather(results):
    # per-core o: (MT, P, NSH) -> (MTOT, NSH); concat cores along N
    full = np.concatenate(
        [r["o"].reshape(MTOT, NSH) for r in results], axis=1
    )  # (MTOT, N)
    outs = []
    off = 0
    for m in MS:
        outs.append(full[off : off + m])
        off += m
    return tuple(outs)


def kernel(**inputs):
    a_list = [np.asarray(inputs[f"a{i}"], dtype=np.float32) for i in range(len(MS))]
    b_list = [np.asarray(inputs[f"b{i}"], dtype=np.float32) for i in range(len(MS))]
    in_maps = _prep_inputs(a_list, b_list)
    nc = _build()
    res = run_bass_kernel_spmd(nc, in_maps, core_ids=list(range(NCORES)))
    return _gather(res.results)


# Exposed for test harness: run with tracing enabled.
def kernel_traced(inputs, profile_dir, trace_cores=None):
    a_list = [np.asarray(inputs[f"a{i}"], dtype=np.float32) for i in range(len(MS))]
    b_list = [np.asarray(inputs[f"b{i}"], dtype=np.float32) for i in range(len(MS))]
    in_maps = _prep_inputs(a_list, b_list)
    nc = _build()
    res = run_bass_kernel_spmd(
        nc,
        in_maps,
        core_ids=list(range(NCORES)),
        trace=True,
        tmpdir=profile_dir,
        trace_cores=trace_cores if trace_cores is not None else list(range(NCORES)),
    )
    return _gather(res.results), res


# revision 2
# speedup vs baseline: 1.0155x; 1.0155x over previous
"""Grouped GEMM (MoE expert-parallel) Trainium2 kernel.

8 independent problems: (M_i, 2048) @ (2048, 8192), M_i in MS below.
Sharding: tensor-parallel along N across the 8 cores — every core computes
ALL groups restricted to its own 1024-column slice of N. This is perfectly
load-balanced (each core does sum(M_i) x 2048 x 1024 MACs) and lets a single
SPMD NEFF run on all cores with only the b-slice data differing per core.

Matmuls run in float32r (the tensor engine's fast fp32 mode): measured
~2.4e-4 max rel err vs fp32 on K=2048 dot products, at bf16-rate throughput
(~255 ns per 128x128x512 matmul vs 887 ns for plain fp32).

Host-side pre/post-processing (free — not HW time):
  - A matrices are concatenated, transposed and tiled to [ki, mt, ko, mi]
    so each a-tile DMA is one contiguous 8KB run per partition.
  - B slices are tiled to [g, ki, ko, n] (64KB contiguous per partition).
  - Outputs come back as (mt, mi, n) per core and are re-assembled on host.
"""
import numpy as np

import concourse.bacc as bacc
import concourse.mybir as mybir
import concourse.tile as tile
from concourse.bass_utils import run_bass_kernel_spmd

MS = [4096, 1024, 2048, 3072, 512, 1536, 2560, 768]
K = 2048
N = 8192
NCORES = 8
NSH = N // NCORES  # 1024 columns per core
P = 128
KO = K // P  # 16
MTOT = sum(MS)  # 15616
MT = MTOT // P  # 122 m-tiles
NH = NSH // 512  # 2 psum-tile halves per m-tile

_NC_CACHE = {}


def _build(mode="f32r"):
    if mode in _NC_CACHE:
        return _NC_CACHE[mode]
    in_dt = {"f32r": mybir.dt.float32r, "f32": mybir.dt.float32}[mode]

    nc = bacc.Bacc("TRN2", target_bir_lowering=False)
    aT = nc.dram_tensor("aT", [P, MT, KO, P], in_dt, kind="ExternalInput")
    b = nc.dram_tensor("b", [len(MS), P, KO, NSH], in_dt, kind="ExternalInput")
    o = nc.dram_tensor("o", [MT, P, NSH], mybir.dt.float32, kind="ExternalOutput")

    # m-tile index ranges per group
    g_tiles = []
    t0 = 0
    for m in MS:
        nt = m // P
        g_tiles.append(range(t0, t0 + nt))
        t0 += nt
    assert t0 == MT

    with tile.TileContext(nc) as tc:
        with (
            tc.tile_pool(name="bp", bufs=2) as bp,
            tc.tile_pool(name="ap", bufs=5) as ap,
            tc.tile_pool(name="op", bufs=4) as op,
            tc.tile_pool(name="pp", bufs=8, space="PSUM") as pp,
        ):
            for g in range(len(MS)):
                b_sb = bp.tile([P, KO, NSH], in_dt, tag="b")
                # b loads go on the ACT HWDGE queue (separate from a/o on SP)
                # and in 4 chunks, so a-tile DMAs are not head-of-line blocked
                # behind a monolithic 8MB transfer.
                for kc in range(4):
                    nc.scalar.dma_start(
                        b_sb[:, 4 * kc : 4 * kc + 4, :], b[g, :, 4 * kc : 4 * kc + 4, :]
                    )
                for mt in g_tiles[g]:
                    a_sb = ap.tile([P, KO, P], in_dt, tag="a")
                    nc.sync.dma_start(a_sb, aT[:, mt])
                    o_sb = op.tile([P, NSH], mybir.dt.float32, tag="o")
                    for nh in range(NH):
                        ps = pp.tile([P, 512], mybir.dt.float32, tag="ps")
                        for ko in range(KO):
                            nc.tensor.matmul(
                                ps,
                                a_sb[:, ko, :],
                                b_sb[:, ko, nh * 512 : (nh + 1) * 512],
                                start=(ko == 0),
                                stop=(ko == KO - 1),
                            )
                        nc.vector.tensor_copy(o_sb[:, nh * 512 : (nh + 1) * 512], ps)
                    nc.sync.dma_start(o[mt], o_sb)

    nc.compile()
    _NC_CACHE[mode] = nc
    return nc


def _prep_inputs(a_list, b_list):
    # A: concat groups -> (MTOT, K); tile to [ki, mt, ko, mi]
    a_cat = np.concatenate(a_list, axis=0)
    aT = np.ascontiguousarray(
        a_cat.reshape(MT, P, KO, P).transpose(3, 0, 2, 1)
    )  # [ki, mt, ko, mi]
    in_maps = []
    for c in range(NCORES):
        bc = np.stack(
            [
                np.ascontiguousarray(
                    bg[:, c * NSH : (c + 1) * NSH]
                    .reshape(KO, P, NSH)
                    .transpose(1, 0, 2)
                )
                for bg in b_list
            ],
            axis=0,
        )  # (G, ki, ko, n)
        in_maps.append({"aT": aT, "b": bc})
    return in_maps


def _gather(results):
    # per-core o: (MT, P, NSH) -> (MTOT, NSH); concat cores along N
    full = np.concatenate(
        [r["o"].reshape(MTOT, NSH) for r in results], axis=1
    )  # (MTOT, N)
    outs = []
    off = 0
    for m in MS:
        outs.append(full[off : off + m])
        off += m
    return tuple(outs)


def kernel(**inputs):
    a_list = [np.asarray(inputs[f"a{i}"], dtype=np.float32) for i in range(len(MS))]
    b_list = [np.asarray(inputs[f"b{i}"], dtype=np.float32) for i in range(len(MS))]
    in_maps = _prep_inputs(a_list, b_list)
    nc = _build()
    res = run_bass_kernel_spmd(nc, in_maps, core_ids=list(range(NCORES)))
    return _gather(res.results)


# Exposed for test harness: run with tracing enabled.
def kernel_traced(inputs, profile_dir, trace_cores=None):
    a_list = [np.asarray(inputs[f"a{i}"], dtype=np.float32) for i in range(len(MS))]
    b_list = [np.asarray(inputs[f"b{i}"], dtype=np.float32) for i in range(len(MS))]
    in_maps = _prep_inputs(a_list, b_list)
    nc = _build()
    res = run_bass_kernel_spmd(
        nc,
        in_maps,
        core_ids=list(range(NCORES)),
        trace=True,
        tmpdir=profile_dir,
        trace_cores=trace_cores if trace_cores is not None else list(range(NCORES)),
    )
    return _gather(res.results), res
